# revision 21
# baseline (speedup 1.0000x reference)
"""CrossNetMix (MoE cross-network) Trainium2 kernel.

Math per layer (reference):
    gates = softmax(x_l @ gate_w.T)                  # [B, E]
    v     = tanh(x_l @ V[l])                         # [B, E, R]  (per expert)
    v2    = tanh(v @ C[l].T)                         # [B, E, R]  (per expert)
    uv    = v2 @ U[l].T                              # [B, E, D]  (per expert)
    x_l   = x0 * (sum_e gates_e * uv_e + bias[l]) + x_l

Kernel strategy (per core, batch data-parallel over 8 cores):
  - activations kept feature-major ("transposed", [feature, batch]) in SBUF
    so every matmul contracts along the partition dim with weights stationary
  - gate softmax: exp on ACT, sum/replication via tiny PE matmuls with
    ones/one-hot matrices, so gates fold into v2 rows before the U matmul
    (softmax weights sum to 1, so bias passes straight through the mix)
  - scale-space state: x_l = x0 * s_l with s_{l+1} = s_l + (moe_mix + bias),
    so the per-m-tile layer update is ONE vector add (PSUM + SBUF -> SBUF)
    plus one gpsimd (Pool-engine) multiply x_{l+1} = x0 * s_{l+1}; this
    halves the DVE elementwise load of the naive mul+add residual form and
    moves the other half to the otherwise-idle Pool engine
  - float32r everywhere on the PE: full bf16-rate with ~1.5e-4 matmul rel-err
  - input/output layout change via PE transposes (fp32 DMA transpose is not
    supported by the xbar)
"""

import sys

for _p in ("/opt/trn_rl_repo", "/root/.axon_site/_ro/trn_rl_repo"):
    if _p not in sys.path:
        sys.path.insert(0, _p)

import numpy as np
from contextlib import ExitStack

import concourse.bass as bass
import concourse.tile as tile
import concourse.mybir as mybir
from concourse.bass_utils import run_bass_kernel_spmd

B, D, R, E, L = 16384, 1024, 64, 4, 3
N_CORES = 8
BC = B // N_CORES          # 2048 rows per core
CH = 512                   # batch columns per processing chunk
ER = E * R                 # 256

f32 = mybir.dt.float32
f32r = mybir.dt.float32r
bf16 = mybir.dt.bfloat16
AF = mybir.ActivationFunctionType

MAX_WAITS = 1


def split_sync_waits(nc, max_waits=MAX_WAITS):
    """Walrus in this container rejects >1 sync-wait per instruction; spread
    extra waits onto preceding same-engine NoOps."""
    ctr = 0
    for f in nc.m.functions:
        for blk in f.blocks:
            insts = list(blk.instructions)
            new = []
            for inst in insts:
                si = inst.sync_info
                if si is not None and len(si.on_wait) > max_waits:
                    waits = list(si.on_wait)
                    over = waits[:-max_waits]
                    keep = waits[-max_waits:]
                    for i in range(0, len(over), max_waits):
                        chunk = over[i:i + max_waits]
                        nop = mybir.InstNoOp(
                            name=f"waitsplit_{ctr}",
                            sync_info=mybir.SyncInfo(on_wait=chunk, on_update=[]),
                            bass_nofuse=True,
                            engine=inst.engine,
                        )
                        ctr += 1
                        new.append(nop)
                    si.on_wait = keep
                    inst.sync_info = si
                new.append(inst)
            blk.instructions = new
    return ctr


def build_nc(bc=BC, ch=None, use_bias=False, layer_reps=1, loop_reps=1, grep_direct=True):
    """Two chunks of 512 batch-columns are software-pipelined through the
    layer loop (loop order: super-chunk -> layer -> chunk) so the serial
    gates->V->C->U dependency chain of one chunk hides behind the other
    chunk's matmuls. Weights arrive host-packed in SBUF layout, one DMA
    per tensor; the first chunk's input DMA is issued before the weights."""
    NW = 512                     # matmul / elementwise tile width
    assert bc % (2 * NW) == 0
    n_sc = bc // (2 * NW)        # super-chunks (pairs of 512-col chunks)

    nc = bass.Bass("TRN2", target_bir_lowering=False, debug=False)

    in_ap = nc.dram_tensor("inputs", [bc, D], f32, kind="ExternalInput").ap()
    vcat_ap = nc.dram_tensor("vcat", [128, L * 8 * ER], bf16, kind="ExternalInput").ap()
    ucat_ap = nc.dram_tensor("ucat", [128, L * 2 * D], bf16, kind="ExternalInput").ap()
    cbd_ap = nc.dram_tensor("cbd", [128, L * 2 * 128], bf16, kind="ExternalInput").ap()
    gt_ap = nc.dram_tensor("gt", [128, 8 * E], bf16, kind="ExternalInput").ap()
    oneh_ap = nc.dram_tensor("oneh", [E, ER], bf16, kind="ExternalInput").ap()
    ones41_ap = nc.dram_tensor("ones41", [E, 1], bf16, kind="ExternalInput").ap()
    ones14_ap = nc.dram_tensor("ones14", [1, E], bf16, kind="ExternalInput").ap()
    ident_ap = nc.dram_tensor("ident", [128, 128], f32r, kind="ExternalInput").ap()
    identb_ap = nc.dram_tensor("identb", [128, 128], bf16, kind="ExternalInput").ap()
    bias_ap = nc.dram_tensor("bias", [128, L * 8], f32, kind="ExternalInput").ap()
    biasp1_ap = nc.dram_tensor("biasp1", [128, L * 8], f32, kind="ExternalInput").ap()
    out_ap = nc.dram_tensor("out", [bc, D], f32, kind="ExternalOutput").ap()

    with tile.TileContext(nc) as tc, ExitStack() as ctx:
        const = ctx.enter_context(tc.tile_pool(name="const", bufs=1))
        state = ctx.enter_context(tc.tile_pool(name="state", bufs=1))
        xin_p = ctx.enter_context(tc.tile_pool(name="xin", bufs=2))
        v_p = ctx.enter_context(tc.tile_pool(name="vp", bufs=3))
        v2g_p = ctx.enter_context(tc.tile_pool(name="v2gp", bufs=3))
        g4_p = ctx.enter_context(tc.tile_pool(name="g4p", bufs=2))
        grs_p = ctx.enter_context(tc.tile_pool(name="grsp", bufs=3))
        ost_p = ctx.enter_context(tc.tile_pool(name="ostp", bufs=2))
        mm_ps = ctx.enter_context(tc.tile_pool(name="mmps", bufs=3, space="PSUM"))
        mmb_ps = ctx.enter_context(tc.tile_pool(name="mmbps", bufs=2, space="PSUM"))
        grep_ps = ctx.enter_context(tc.tile_pool(name="grepps", bufs=2, space="PSUM"))
        sm_ps = ctx.enter_context(tc.tile_pool(name="smps", bufs=1, space="PSUM"))

        ident_sb = const.tile([128, 128], f32r)
        nc.sync.dma_start(ident_sb[:], ident_ap[:])
        identb_sb = const.tile([128, 128], bf16)
        nc.sync.dma_start(identb_sb[:], identb_ap[:])

        # state: feature-major, both pipelined chunks side by side:
        # column = m*1024 + c2*512 + b
        sT = state.tile([128, 8 * 2 * NW], f32r)    # s_l  (x_l = x0 * s_l)
        xcur = state.tile([128, 8 * 2 * NW], bf16)  # materialized x_l, l >= 1
        x0T = state.tile([128, 8 * 2 * NW], bf16)

        def xsl(t, m, c2):
            return t[:, m * 2 * NW + c2 * NW:m * 2 * NW + (c2 + 1) * NW]

        # stage-in is split: the HBM DMA (on the ACT HWDGE ring, so it never
        # queues behind output DMAs on the SP ring) is issued early, the PE
        # transposes into x0T run later, once x0T's previous super-chunk
        # readers are done.
        xin_tiles = {}

        def stage_in_dma(sc, c2):
            # SWDGE cast-DMA: HBM fp32 -> SBUF bf16, batch-major
            xin = xin_p.tile([128, 4 * D], bf16)
            row0 = (sc * 2 + c2) * NW
            nc.gpsimd.dma_start(
                xin[:, :2 * D],
                in_ap[row0:row0 + 256, :].rearrange("(j p) d -> p j d", p=128))
            nc.gpsimd.dma_start(
                xin[:, 2 * D:],
                in_ap[row0 + 256:row0 + NW, :].rearrange("(j p) d -> p j d", p=128))
            xin_tiles[(sc, c2)] = xin

        def stage_in_tp(sc, c2):
            # xbar DMA transpose: batch-major bf16 -> feature-major x0T slices
            xin = xin_tiles.pop((sc, c2))
            x0r = x0T[:].rearrange("p (m w) -> p m w", m=8)
            for j in range(4):
                nc.sync.dma_start(
                    x0r[:, :, c2 * NW + j * 128:c2 * NW + (j + 1) * 128],
                    xin[:, j * D:(j + 1) * D], transpose=True)

        def stage_in(sc, c2):
            stage_in_dma(sc, c2)
            stage_in_tp(sc, c2)

        # first chunk's input ahead of the weight bulk
        if loop_reps == 1:
            stage_in(0, 0)

        gt_sb = const.tile([128, 8 * E], bf16)
        nc.sync.dma_start(gt_sb[:], gt_ap[:])
        oneh_sb = const.tile([E, ER], bf16)
        nc.sync.dma_start(oneh_sb[:], oneh_ap[:])
        ones41_sb = const.tile([E, 1], bf16)
        nc.sync.dma_start(ones41_sb[:], ones41_ap[:])
        ones14_sb = const.tile([1, E], bf16)
        nc.sync.dma_start(ones14_sb[:], ones14_ap[:])
        vcat_sb = const.tile([128, L * 8 * ER], bf16)
        ucat_sb = const.tile([128, L * 2 * D], bf16)
        cbd_sb = const.tile([128, L * 2 * 128], bf16)
        for l in range(L):
            nc.sync.dma_start(vcat_sb[:, l * 8 * ER:(l + 1) * 8 * ER],
                              vcat_ap[:, l * 8 * ER:(l + 1) * 8 * ER])
            nc.sync.dma_start(cbd_sb[:, l * 256:(l + 1) * 256],
                              cbd_ap[:, l * 256:(l + 1) * 256])
            nc.sync.dma_start(ucat_sb[:, l * 2 * D:(l + 1) * 2 * D],
                              ucat_ap[:, l * 2 * D:(l + 1) * 2 * D])
            if l == 0 and loop_reps == 1:
                stage_in(0, 1)
        bias_sb = const.tile([128, L * 8], f32)
        biasp1_sb = const.tile([128, L * 8], f32)
        if use_bias:
            nc.sync.dma_start(bias_sb[:], bias_ap[:])
            nc.sync.dma_start(biasp1_sb[:], biasp1_ap[:])

        def emit_body():
            for sc in range(n_sc):
                for li in range(L * layer_reps):
                    l = li % L
                    X = x0T if li == 0 else xcur
                    if li == 0 and sc + 1 < n_sc:
                        # prefetch next super-chunk's input while this one computes
                        stage_in_dma(sc + 1, 0)
                        stage_in_dma(sc + 1, 1)
                    exp4s, recs, g4s, vts, grsbs, v2gs = {}, {}, {}, {}, {}, {}
                    # gate logits
                    for c2 in range(2):
                        lp = sm_ps.tile([E, NW], f32, tag="sm")
                        for k in range(8):
                            nc.tensor.matmul(lp[:], gt_sb[:, k * E:(k + 1) * E],
                                             xsl(X, k, c2),
                                             start=(k == 0), stop=(k == 7))
                        e4 = g4_p.tile([E, NW], bf16, tag="exp4")
                        nc.scalar.activation(e4[:], lp[:], AF.Exp)
                        exp4s[c2] = e4
                    # V mt=0
                    for c2 in range(2):
                        vp_ = mm_ps.tile([128, NW], f32, tag="mm")
                        for k in range(8):
                            nc.tensor.matmul(
                                vp_[:],
                                vcat_sb[:, (l * 8 + k) * ER:(l * 8 + k) * ER + 128],
                                xsl(X, k, c2), start=(k == 0), stop=(k == 7))
                        vt = v_p.tile([128, NW], bf16, tag="v")
                        nc.scalar.activation(vt[:], vp_[:], AF.Tanh)
                        vts[(c2, 0)] = vt
                    # gate sum + recip
                    for c2 in range(2):
                        sp_ = sm_ps.tile([1, NW], f32, tag="sm")
                        nc.tensor.matmul(sp_[:], ones41_sb[:], exp4s[c2][:])
                        rc = g4_p.tile([1, NW], bf16, tag="rec")
                        with nc.allow_low_precision(reason="float32r is 4-byte"):
                            nc.vector.reciprocal(rc[:], sp_[:])
                        recs[c2] = rc
                    # V mt=1
                    for c2 in range(2):
                        vp_ = mm_ps.tile([128, NW], f32, tag="mm")
                        for k in range(8):
                            nc.tensor.matmul(
                                vp_[:],
                                vcat_sb[:, (l * 8 + k) * ER + 128:(l * 8 + k) * ER + 256],
                                xsl(X, k, c2), start=(k == 0), stop=(k == 7))
                        vt = v_p.tile([128, NW], bf16, tag="v")
                        nc.scalar.activation(vt[:], vp_[:], AF.Tanh)
                        vts[(c2, 1)] = vt
                    # gate normalize + replicate (PSUM -> ACT evict to SBUF)
                    for c2 in range(2):
                        r4 = sm_ps.tile([E, NW], f32, tag="sm")
                        nc.tensor.matmul(r4[:], ones14_sb[:], recs[c2][:])
                        g4 = exp4s[c2]   # in-place: g4 = exp4 * r4
                        nc.vector.tensor_mul(g4[:], exp4s[c2][:], r4[:])
                        g4s[c2] = g4
                        for mt in range(2):
                            gp = grep_ps.tile([128, NW], f32, tag="grep")
                            nc.tensor.matmul(gp[:], oneh_sb[:, mt * 128:(mt + 1) * 128],
                                             g4[:])
                            if grep_direct:
                                grsbs[(c2, mt)] = gp
                            else:
                                gs = grs_p.tile([128, NW], f32r, tag="grs")
                                nc.scalar.activation(gs[:], gp[:], AF.Copy)
                                grsbs[(c2, mt)] = gs
                    # C + gate weighting
                    for c2 in range(2):
                        for mt in range(2):
                            cp = mm_ps.tile([128, NW], f32, tag="mm")
                            nc.tensor.matmul(
                                cp[:],
                                cbd_sb[:, (l * 2 + mt) * 128:(l * 2 + mt + 1) * 128],
                                vts[(c2, mt)][:])
                            v2 = v_p.tile([128, NW], bf16, tag="v2")
                            nc.scalar.activation(v2[:], cp[:], AF.Tanh)
                            v2g = v2g_p.tile([128, NW], bf16, tag="v2g")
                            nc.vector.tensor_mul(v2g[:], v2[:], grsbs[(c2, mt)][:])
                            v2gs[(c2, mt)] = v2g
                    # U + scale-space state update: s_{l+1} = s_l + wp (+bias),
                    # x_{l+1} = x0 * s_{l+1} (Pool engine).  On the last layer,
                    # immediately follow each chunk's update with its stage-out
                    # transposes and the next super-chunk's stage-in so the PE
                    # transpose burst overlaps the other chunk's U-phase tail.
                    last_layer = (li == L * layer_reps - 1)
                    for c2 in range(2):
                        for m in range(8):
                            wp = mm_ps.tile([128, NW], f32, tag="mm")
                            for kt in range(2):
                                nc.tensor.matmul(
                                    wp[:],
                                    ucat_sb[:, (l * 2 + kt) * D + m * 128:
                                            (l * 2 + kt) * D + (m + 1) * 128],
                                    v2gs[(c2, kt)][:], start=(kt == 0), stop=(kt == 1))
                            s_sl = xsl(sT, m, c2)
                            if li == 0:
                                if use_bias:
                                    nc.vector.tensor_scalar_add(
                                        s_sl, wp[:],
                                        biasp1_sb[:, l * 8 + m:l * 8 + m + 1])
                                else:
                                    nc.vector.tensor_scalar_add(s_sl, wp[:], 1.0)
                            else:
                                if use_bias:
                                    nc.vector.scalar_tensor_tensor(
                                        s_sl, wp[:],
                                        bias_sb[:, l * 8 + m:l * 8 + m + 1],
                                        s_sl, op0=mybir.AluOpType.add,
                                        op1=mybir.AluOpType.add)
                                else:
                                    nc.vector.tensor_add(s_sl, wp[:], s_sl)
                            nc.gpsimd.tensor_mul(
                                xsl(xcur, m, c2), xsl(x0T, m, c2), s_sl)
                    if last_layer:
                        # both chunks' U-phases are already in the PE queue, so
                        # chunk 1's matmuls hide chunk 0's add->mul chain before
                        # its stage-out transposes start
                        for c2 in range(2):
                            for j in range(4):
                                ost = ost_p.tile([128, D], f32, tag="ost")
                                op_ = mmb_ps.tile([128, D], bf16, tag="mmb")
                                for m in range(8):
                                    nc.tensor.matmul(
                                        op_[:, m * 128:(m + 1) * 128],
                                        xcur[:, m * 2 * NW + c2 * NW + j * 128:
                                             m * 2 * NW + c2 * NW + (j + 1) * 128],
                                        identb_sb[:], is_transpose=True,
                                        start=(m == 0), stop=(m == 7))
                                nc.scalar.activation(ost[:], op_[:], AF.Copy)
                                row0 = (sc * 2 + c2) * NW + j * 128
                                nc.sync.dma_start(out_ap[row0:row0 + 128, :], ost[:])
                        if sc + 1 < n_sc:
                            for c2 in range(2):
                                stage_in_tp(sc + 1, c2)


        from contextlib import nullcontext
        loop_cm = tc.For_i(0, loop_reps, 1) if loop_reps > 1 else nullcontext()
        with loop_cm:
            if loop_reps > 1:
                stage_in(0, 0)
                stage_in(0, 1)
            emit_body()

    return nc


def prep_weights(U, V, C, gate_w, bias):
    U = np.asarray(U, dtype=np.float32)
    V = np.asarray(V, dtype=np.float32)
    C = np.asarray(C, dtype=np.float32)
    gate_w = np.asarray(gate_w, dtype=np.float32)
    bias = np.asarray(bias, dtype=np.float32)
    # vcat[l, d, e*R+r] = V[l, e, d, r]; packed [128, (l*8+k)*256 + er]
    vcat = V.transpose(0, 2, 1, 3).reshape(L, D, ER)
    vcat_pack = np.ascontiguousarray(
        vcat.reshape(L, 8, 128, ER).transpose(2, 0, 1, 3).reshape(128, L * 8 * ER))
    # ucat[l, e*R+r, d] = U[l, e, d, r]; packed [128, (l*2+kt)*1024 + d]
    ucat = U.transpose(0, 1, 3, 2).reshape(L, ER, D)
    ucat_pack = np.ascontiguousarray(
        ucat.reshape(L, 2, 128, D).transpose(2, 0, 1, 3).reshape(128, L * 2 * D))
    cbd = np.zeros((L, 2, 128, 128), dtype=np.float32)
    for l in range(L):
        for p in range(2):
            cbd[l, p, :R, :R] = C[l, 2 * p].T
            cbd[l, p, R:, R:] = C[l, 2 * p + 1].T
    cbd_pack = np.ascontiguousarray(
        cbd.transpose(2, 0, 1, 3).reshape(128, L * 2 * 128))
    gt = gate_w.T  # [D, E]
    gt_pack = np.ascontiguousarray(
        gt.reshape(8, 128, E).transpose(1, 0, 2).reshape(128, 8 * E))
    bias_pack = np.ascontiguousarray(
        bias.reshape(L, 8, 128).transpose(2, 0, 1).reshape(128, L * 8))
    oneh = np.kron(np.eye(E), np.ones((1, R))).astype(np.float32)
    ones41 = np.ones((E, 1), dtype=np.float32)
    ones14 = np.ones((1, E), dtype=np.float32)
    ident = np.eye(128, dtype=np.float32)
    bf = mybir.dt.np(mybir.dt.bfloat16)
    return dict(vcat=vcat_pack.astype(bf), ucat=ucat_pack.astype(bf),
                cbd=cbd_pack.astype(bf), gt=gt_pack.astype(bf),
                oneh=oneh.astype(bf), ones41=ones41.astype(bf),
                ones14=ones14.astype(bf), ident=ident,
                identb=ident.astype(bf),
                bias=bias_pack, biasp1=bias_pack + 1.0)


_NC_CACHE = {}


def get_nc(bc=BC, use_bias=False, loop_reps=1, grep_direct=True):
    key = (bc, use_bias, loop_reps, grep_direct)
    if key not in _NC_CACHE:
        nc = build_nc(bc, use_bias=use_bias, loop_reps=loop_reps,
                      grep_direct=grep_direct)
        split_sync_waits(nc)
        _NC_CACHE[key] = nc
    return _NC_CACHE[key]


def make_in_maps(inputs, U, V, C, gate_w, bias):
    inputs = np.ascontiguousarray(np.asarray(inputs, dtype=np.float32))
    w = prep_weights(U, V, C, gate_w, bias)
    in_maps = []
    for c in range(N_CORES):
        m = {"inputs": inputs[c * BC:(c + 1) * BC]}
        m.update(w)
        in_maps.append(m)
    return in_maps


def kernel(inputs, U, V, C, gate_w, bias):
    use_bias = bool(np.any(np.asarray(bias)))
    nc = get_nc(use_bias=use_bias)
    in_maps = make_in_maps(inputs, U, V, C, gate_w, bias)
    res = run_bass_kernel_spmd(nc, in_maps, list(range(N_CORES)))
    out = np.concatenate([res.results[c]["out"] for c in range(N_CORES)], axis=0)
    return out.astype(np.float32)

